# revision 8
# baseline (speedup 1.0000x reference)
"""GATv2 message-passing kernel for Trainium2 (8 NeuronCores, Bass/Tile).

Strategy (dst-range sharding, not plain edge sharding):
  - Nodes are split into 8 contiguous ranges; core c owns dst range
    [c*6250, (c+1)*6250) and ALL edges pointing into it.
  - Each core computes the h_src/h_dst projections for its own node range
    (feat slice @ w), then an AllGather replicates the full h_src table
    (needed for per-edge src gathers). h_dst stays local (only local dst
    rows are ever needed).
  - Softmax without the segment-max stabilizer: alpha = exp(s)/sum(exp(s))
    is mathematically identical to the max-subtracted version and scores
    here are O(+-10), so exp() cannot overflow. This fuses the whole edge
    phase into ONE pass: num[n] = sum_e exp(s_e) * h_src[src_e],
    den[n] = sum_e exp(s_e), rst = num/den.
  - Edges are grouped by destination "block" (128 consecutive dst nodes)
    on the host and padded to a fixed grid of K 128-edge chunks per block.
    Each chunk's segment-sum becomes ONE 128x128x260 matmul with a 0/1
    selection matrix S^T[t, m] = (dstoff_t == m), accumulated in PSUM
    across the block's chunks.  The denominator rides along as 4 extra
    columns (w = exp(score) per head).
  - Epilogue per block: rst = num * (1/den) per head, transpose, fc matmul,
    + bias, DMA out.  gat_bias is folded into b_fc on the host
    (b_fc_eff = b_fc + gat_bias @ w_fc) since z = (num/den + gat_bias) @ w_fc + b_fc.

The full output is assembled host-side from the 8 per-core dst-range slices.
"""

import os
import sys

import numpy as np


def _ensure_concourse():
    try:
        import concourse  # noqa: F401
    except Exception:
        for p in ("/opt/trn_rl_repo", "/root/.axon_site/_ro/trn_rl_repo"):
            if os.path.isdir(p):
                sys.path.insert(0, p)
                break


_ensure_concourse()

import concourse.bass as bass  # noqa: E402
import concourse.mybir as mybir  # noqa: E402
import concourse.tile as tile  # noqa: E402
from concourse import bacc  # noqa: E402

F32 = mybir.dt.float32
I32 = mybir.dt.int32
I16 = mybir.dt.int16
AX = mybir.AxisListType
ALU = mybir.AluOpType
ACTF = mybir.ActivationFunctionType


class GATCfg:
    def __init__(self, N=50000, E=800000, IN=256, H=4, D=64, DOUT=64, NC=8,
                 SUPER=8, GCH=64, neg_slope=0.2,
                 use_act_lrelu=False, use_f32r=False):
        self.N, self.E, self.IN, self.H, self.D, self.DOUT, self.NC = N, E, IN, H, D, DOUT, NC
        self.SUPER, self.GCH, self.neg_slope = SUPER, GCH, neg_slope
        self.use_act_lrelu = use_act_lrelu
        self.use_f32r = use_f32r
        self.HD = H * D
        self.MSG = self.HD + H          # msg columns + per-head exp(score)
        assert N % NC == 0
        self.NPC = N // NC              # real nodes per core
        self.NBLK = (self.NPC + 127) // 128
        self.NPC_PAD = self.NBLK * 128  # padded nodes per core
        self.NFULL = NC * self.NPC_PAD  # padded global h_src table rows
        assert GCH % SUPER == 0
        assert self.HD % 128 == 0 and self.IN % 128 == 0


SPLIT = 32768  # dma_gather idx is int16: table rows >= SPLIT go to view B


def preprocess(edge_index, cfg: GATCfg):
    """Sort edges by dst, partition by owning core, grid them into
    [NBLK x (K_A + K_B) x 128] slots per core (pad with sentinels).

    Each destination block's edges are split into class A (src row in the
    padded h_src table < SPLIT) and class B (>= SPLIT) so src indices fit
    dma_gather's int16; A chunks come first, then B chunks."""
    src = np.asarray(edge_index[0], dtype=np.int64)
    dst = np.asarray(edge_index[1], dtype=np.int64)
    order = np.argsort(dst, kind="stable")
    src_s, dst_s = src[order], dst[order]

    core = dst_s // cfg.NPC
    loc = dst_s - core * cfg.NPC          # dst row within core range
    blk = loc // 128
    gb = core * cfg.NBLK + blk            # global block id
    # padded global src index into the AllGather'd h_src table
    srcp = (src_s // cfg.NPC) * cfg.NPC_PAD + (src_s % cfg.NPC)
    isB = (srcp >= SPLIT).astype(np.int64)

    ncell = cfg.NC * cfg.NBLK
    cell = gb * 2 + isB                   # (block, class) cell id
    cnt = np.bincount(cell, minlength=ncell * 2)
    cntA, cntB = cnt[0::2], cnt[1::2]
    K_A = int(np.ceil(cntA.max() / 128)) if cntA.max() > 0 else 0
    K_B = int(np.ceil(cntB.max() / 128)) if cntB.max() > 0 else 0
    K_A = max(K_A, 1)                     # dummy chunks are class A
    K = K_A + K_B

    # rank of each edge within its (block, class) cell
    order2 = np.argsort(cell, kind="stable")
    inv2 = np.empty_like(order2)
    inv2[order2] = np.arange(len(order2))
    starts = np.zeros(ncell * 2 + 1, dtype=np.int64)
    np.cumsum(cnt, out=starts[1:])
    r = inv2 - starts[cell]
    slot_in_core = blk * (K * 128) + isB * (K_A * 128) + r

    nchunk_real = cfg.NBLK * K
    NCHUNK = ((nchunk_real + cfg.GCH - 1) // cfg.GCH) * cfg.GCH
    NG = NCHUNK // cfg.GCH
    SLOTS = NCHUNK * 128

    idx16_all = np.where(isB == 1, srcp - SPLIT, srcp).astype(np.int16)

    def wrap16(a):
        # [NCHUNK, 128] -> [NG, 128, GCH*8] wrapped for dma_gather:
        # chunk c (col block 8c..8c+8 of its group), idx i=(c-c0)*128+p of a
        # call lands at [p%16 + 16k, 8*(c-c0) + p//16].
        w = a.reshape(NCHUNK, 8, 16).transpose(0, 2, 1)   # [c, 16, 8]
        w = w.reshape(NG, cfg.GCH, 16, 8).transpose(0, 2, 1, 3)  # [g,16,GCH,8]
        w = w.reshape(NG, 16, cfg.GCH * 8)
        return np.ascontiguousarray(np.tile(w, (1, 8, 1)))  # [NG,128,GCH*8]

    per_core = []
    for c in range(cfg.NC):
        m = core == c
        sl = slot_in_core[m]
        sidx16 = np.zeros(SLOTS, np.int16)
        didx16 = np.zeros(SLOTS, np.int16)
        dst_off = np.full(SLOTS, -1.0, np.float32)
        sidx16[sl] = idx16_all[m]
        didx16[sl] = loc[m].astype(np.int16)
        dst_off[sl] = (loc[m] - blk[m] * 128).astype(np.float32)

        def grid(a):
            # [NCHUNK,128] -> [NG, 128, GCH]  (chunk j of group g at [g,:,j])
            return np.ascontiguousarray(
                a.reshape(NG, cfg.GCH, 128).transpose(0, 2, 1))

        per_core.append(dict(
            sidx16w=wrap16(sidx16.reshape(NCHUNK, 128)),
            didx16w=wrap16(didx16.reshape(NCHUNK, 128)),
            dst_off=grid(dst_off)))
    return dict(K=K, K_A=K_A, K_B=K_B, NCHUNK=NCHUNK, NG=NG,
                per_core=per_core)


def build(cfg: GATCfg, K: int, NCHUNK: int, K_A: int = None):
    NC, H, D, HD, DOUT, IN = cfg.NC, cfg.H, cfg.D, cfg.HD, cfg.DOUT, cfg.IN
    SUPER, GCH, MSG = cfg.SUPER, cfg.GCH, cfg.MSG
    if K_A is None:
        K_A = K
    NG = NCHUNK // GCH
    KIN = IN // 128     # contraction tiles for projections
    KHD = HD // 128     # contraction tiles for fc
    mm_dt = mybir.dt.float32r if cfg.use_f32r else F32

    nc = bacc.Bacc("TRN2", target_bir_lowering=False, debug=False,
                   num_devices=NC)

    def inp(name, shape, dt=F32):
        return nc.dram_tensor(name, shape, dt, kind="ExternalInput").ap()

    featT = inp("featT", [IN, cfg.NPC_PAD])
    w_src = inp("w_src", [IN, HD])
    w_dst = inp("w_dst", [IN, HD])
    b_src_rep = inp("b_src_rep", [128, HD])
    b_dst_rep = inp("b_dst_rep", [128, HD])
    attn_rep = inp("attn_rep", [128, SUPER * HD])
    iota_rep = inp("iota_rep", [128, SUPER * 128])
    ident = inp("ident", [128, 128])
    w_fc = inp("w_fc", [HD, DOUT])
    b_fc_rep = inp("b_fc_rep", [128, DOUT])
    sidx16w = inp("sidx16w", [NG, 128, GCH * 8], I16)
    didx16w = inp("didx16w", [NG, 128, GCH * 8], I16)
    dst_off = inp("dst_off", [NG, 128, GCH])

    z = nc.dram_tensor("z", [cfg.NPC_PAD, DOUT], F32, kind="ExternalOutput").ap()
    h_src_loc = nc.dram_tensor("h_src_loc", [cfg.NPC_PAD, HD], F32,
                               kind="Internal").ap()
    h_dst_loc = nc.dram_tensor("h_dst_loc", [cfg.NPC_PAD, HD], F32,
                               kind="Internal").ap()
    h_src_full = nc.dram_tensor("h_src_full", [cfg.NFULL, HD], F32,
                                kind="Internal", addr_space="Shared").ap()

    def mdt(ap):
        return ap.bitcast(mm_dt) if cfg.use_f32r else ap

    with tile.TileContext(nc) as tc:
        with (
            tc.tile_pool(name="const", bufs=1) as cp,
            tc.tile_pool(name="proj", bufs=3) as projp,
            tc.tile_pool(name="proj_ps", bufs=2, space="PSUM") as projps,
            tc.tile_pool(name="idxg", bufs=2) as idxp,
            tc.tile_pool(name="edge", bufs=3) as ep,
            tc.tile_pool(name="agg_ps", bufs=2, space="PSUM") as aggps,
            tc.tile_pool(name="epi", bufs=2) as epi,
            tc.tile_pool(name="epi_ps", bufs=2, space="PSUM") as epips,
        ):
            # ---- constants ----
            wsrc_t = cp.tile([128, KIN, HD], F32, tag="wsrc")
            nc.sync.dma_start(out=wsrc_t[:],
                              in_=w_src.rearrange("(k p) c -> p k c", p=128))
            wdst_t = cp.tile([128, KIN, HD], F32, tag="wdst")
            nc.sync.dma_start(out=wdst_t[:],
                              in_=w_dst.rearrange("(k p) c -> p k c", p=128))
            bsrc_t = cp.tile([128, HD], F32, tag="bsrc")
            nc.sync.dma_start(out=bsrc_t[:], in_=b_src_rep[:, :])
            bdst_t = cp.tile([128, HD], F32, tag="bdst")
            nc.sync.dma_start(out=bdst_t[:], in_=b_dst_rep[:, :])
            attn_t = cp.tile([128, SUPER, HD], F32, tag="attn")
            nc.sync.dma_start(out=attn_t[:],
                              in_=attn_rep.rearrange("p (s c) -> p s c", s=SUPER))
            iota_t = cp.tile([128, SUPER, 128], F32, tag="iota")
            nc.sync.dma_start(out=iota_t[:],
                              in_=iota_rep.rearrange("p (s m) -> p s m", s=SUPER))
            ident_t = cp.tile([128, 128], F32, tag="ident")
            nc.sync.dma_start(out=ident_t[:], in_=ident[:, :])
            wfc_t = cp.tile([128, KHD, DOUT], F32, tag="wfc")
            nc.sync.dma_start(out=wfc_t[:],
                              in_=w_fc.rearrange("(k p) c -> p k c", p=128))
            bfc_t = cp.tile([128, DOUT], F32, tag="bfc")
            nc.sync.dma_start(out=bfc_t[:], in_=b_fc_rep[:, :])

            # ---- phase A: projections h = feat @ w + b ----
            featT_v = featT.rearrange("(k p) n -> p k n", p=128)
            for blk in range(cfg.NBLK):
                fT = projp.tile([128, KIN, 128], F32, tag="fT")
                nc.sync.dma_start(
                    out=fT[:], in_=featT_v[:, :, blk * 128:(blk + 1) * 128])
                for tbl, wt, bt, tg in ((h_src_loc, wsrc_t, bsrc_t, "hs"),
                                        (h_dst_loc, wdst_t, bdst_t, "hd")):
                    ps = projps.tile([128, HD], F32)
                    for k in range(KIN):
                        nc.tensor.matmul(out=ps[:], lhsT=mdt(fT[:, k, :]),
                                         rhs=mdt(wt[:, k, :]),
                                         start=(k == 0), stop=(k == KIN - 1))
                    hsb = projp.tile([128, HD], F32, tag=tg)
                    nc.vector.tensor_add(out=hsb[:], in0=ps[:], in1=bt[:])
                    nc.sync.dma_start(out=tbl[blk * 128:(blk + 1) * 128, :],
                                      in_=hsb[:])

            # ---- AllGather h_src across the 8 cores ----
            nc.gpsimd.collective_compute(
                "AllGather", ALU.bypass,
                ins=[h_src_loc[:, :]], outs=[h_src_full[:, :]],
                replica_groups=[list(range(NC))])

            # ---- phase B: edge loop ----
            # chunk class: A chunks gather h_src rows [0, SPLIT), B chunks
            # rows [SPLIT, NFULL) with idx biased by -SPLIT (int16 range)
            hsA = h_src_full[0:min(SPLIT, cfg.NFULL), :]
            hsB = h_src_full[SPLIT:, :] if cfg.NFULL > SPLIT else None

            def chunk_is_B(c):
                return (c % K) >= K_A

            cur_ps = None
            for g in range(NG):
                sidxg = idxp.tile([128, GCH * 8], I16, tag="sidxg")
                nc.sync.dma_start(out=sidxg[:], in_=sidx16w[g, :, :])
                didxg = idxp.tile([128, GCH * 8], I16, tag="didxg")
                nc.sync.dma_start(out=didxg[:], in_=didx16w[g, :, :])
                dstog = idxp.tile([128, GCH], F32, tag="dstog")
                nc.sync.dma_start(out=dstog[:], in_=dst_off[g, :, :])

                for ss in range(GCH // SUPER):
                    cols = slice(ss * SUPER, (ss + 1) * SUPER)
                    base_chunk = g * GCH + ss * SUPER

                    hs = ep.tile([128, SUPER, HD], F32, tag="hs")
                    # gather calls per run of same-class chunks in this super
                    a = 0
                    while a < SUPER:
                        b = a + 1
                        clsB = chunk_is_B(base_chunk + a)
                        while b < SUPER and chunk_is_B(base_chunk + b) == clsB:
                            b += 1
                        tab = hsB if clsB else hsA
                        ic0 = (ss * SUPER + a) * 8
                        ic1 = (ss * SUPER + b) * 8
                        nc.gpsimd.dma_gather(
                            out_ap=hs[:, a:b, :], in_ap=tab,
                            idxs_ap=sidxg[:, ic0:ic1],
                            num_idxs=(b - a) * 128,
                            num_idxs_reg=(b - a) * 128, elem_size=HD)
                        a = b
                    x = ep.tile([128, SUPER, HD], F32, tag="x")
                    nc.gpsimd.dma_gather(
                        out_ap=x[:], in_ap=h_dst_loc[:, :],
                        idxs_ap=didxg[:, ss * SUPER * 8:(ss + 1) * SUPER * 8],
                        num_idxs=SUPER * 128, num_idxs_reg=SUPER * 128,
                        elem_size=HD)
                    # x = hs + hd
                    nc.vector.tensor_add(out=x[:], in0=x[:], in1=hs[:])
                    # x = lrelu(x) = max(0.2*x, x)
                    if cfg.use_act_lrelu:
                        nc.scalar.activation(out=x[:], in_=x[:], func=ACTF.Lrelu,
                                             alpha=cfg.neg_slope)
                    else:
                        nc.vector.scalar_tensor_tensor(
                            out=x[:], in0=x[:], scalar=cfg.neg_slope, in1=x[:],
                            op0=ALU.mult, op1=ALU.max)
                    # x = x * attn (broadcast over supers)
                    nc.vector.tensor_mul(out=x[:], in0=x[:], in1=attn_t[:])
                    # score[p, s, h] = sum_d x[p, s, h, d]
                    score = ep.tile([128, SUPER, H], F32, tag="score")
                    nc.vector.tensor_reduce(
                        out=score[:],
                        in_=x[:].rearrange("p s (h d) -> p s h d", h=H),
                        axis=AX.X, op=ALU.add)
                    msg = ep.tile([128, SUPER, MSG], F32, tag="msg")
                    # w = exp(score) written straight into msg cols HD:HD+H
                    nc.scalar.activation(out=msg[:, :, HD:MSG], in_=score[:],
                                         func=ACTF.Exp)
                    # msg[:, :, :HD] = hs * w (broadcast over d)
                    nc.vector.tensor_mul(
                        out=msg[:, :, 0:HD].rearrange("p s (h d) -> p s h d", h=H),
                        in0=hs[:].rearrange("p s (h d) -> p s h d", h=H),
                        in1=msg[:, :, HD:MSG][:, :, :, None].to_broadcast(
                            [128, SUPER, H, D]))
                    # selection matrix S^T[p, s, m] = (dstoff[p, s] == m)
                    sT = ep.tile([128, SUPER, 128], F32, tag="sT")
                    nc.vector.tensor_tensor(
                        out=sT[:], in0=iota_t[:],
                        in1=dstog[:, cols][:, :, None].to_broadcast(
                            [128, SUPER, 128]),
                        op=ALU.is_equal)

                    for j in range(SUPER):
                        c = base_chunk + j
                        b, cib = c // K, c % K
                        if cib == 0:
                            cur_ps = aggps.tile([128, MSG], F32)
                        last = (cib == K - 1) or (c == NCHUNK - 1)
                        nc.tensor.matmul(out=cur_ps[:], lhsT=mdt(sT[:, j, :]),
                                         rhs=mdt(msg[:, j, :]),
                                         start=(cib == 0), stop=last)
                        if cib == K - 1 and b < cfg.NBLK:
                            # ---- block epilogue ----
                            ps = cur_ps
                            den = epi.tile([128, H], F32, tag="den")
                            nc.vector.tensor_scalar_add(
                                out=den[:], in0=ps[:, HD:MSG], scalar1=1e-30)
                            rec = epi.tile([128, H], F32, tag="rec")
                            nc.vector.reciprocal(out=rec[:], in_=den[:])
                            rst = epi.tile([128, HD], F32, tag="rst")
                            for h in range(H):
                                nc.scalar.mul(rst[:, h * D:(h + 1) * D],
                                              ps[:, h * D:(h + 1) * D],
                                              mul=rec[:, h:h + 1])
                            rT = epi.tile([128, KHD, 128], F32, tag="rT")
                            for k in range(KHD):
                                tps = epips.tile([128, 128], F32, tag="tps")
                                nc.tensor.transpose(
                                    out=tps[:],
                                    in_=rst[:, k * 128:(k + 1) * 128],
                                    identity=ident_t[:])
                                nc.vector.tensor_copy(out=rT[:, k, :],
                                                      in_=tps[:])
                            zps = epips.tile([128, DOUT], F32, tag="zps")
                            for k in range(KHD):
                                nc.tensor.matmul(out=zps[:],
                                                 lhsT=mdt(rT[:, k, :]),
                                                 rhs=mdt(wfc_t[:, k, :]),
                                                 start=(k == 0),
                                                 stop=(k == KHD - 1))
                            zsb = epi.tile([128, DOUT], F32, tag="zsb")
                            nc.vector.tensor_add(out=zsb[:], in0=zps[:],
                                                 in1=bfc_t[:])
                            nc.sync.dma_start(
                                out=z[b * 128:(b + 1) * 128, :], in_=zsb[:])

    nc.compile()
    return nc


def make_in_maps(inputs, cfg: GATCfg, prep):
    """Build the per-core input dicts."""
    feat = np.asarray(inputs["feat"], np.float32)
    w_src = np.ascontiguousarray(np.asarray(inputs["w_src"], np.float32))
    w_dst = np.ascontiguousarray(np.asarray(inputs["w_dst"], np.float32))
    b_src = np.asarray(inputs["b_src"], np.float32)
    b_dst = np.asarray(inputs["b_dst"], np.float32)
    attn = np.asarray(inputs["attn"], np.float32).reshape(-1)  # [HD]
    gat_bias = np.asarray(inputs["gat_bias"], np.float32)
    w_fc = np.ascontiguousarray(np.asarray(inputs["w_fc"], np.float32))
    b_fc = np.asarray(inputs["b_fc"], np.float32)

    b_fc_eff = b_fc + gat_bias @ w_fc
    attn_rep = np.ascontiguousarray(
        np.tile(attn[None, :], (128, cfg.SUPER)))
    iota_rep = np.ascontiguousarray(
        np.tile(np.arange(128, dtype=np.float32)[None, :], (128, cfg.SUPER)))
    ident = np.eye(128, dtype=np.float32)
    b_src_rep = np.ascontiguousarray(np.tile(b_src[None, :], (128, 1)))
    b_dst_rep = np.ascontiguousarray(np.tile(b_dst[None, :], (128, 1)))
    b_fc_rep = np.ascontiguousarray(np.tile(b_fc_eff[None, :], (128, 1)))

    in_maps = []
    for c in range(cfg.NC):
        sl = feat[c * cfg.NPC:(c + 1) * cfg.NPC]
        featT = np.zeros((cfg.IN, cfg.NPC_PAD), np.float32)
        featT[:, :cfg.NPC] = sl.T
        pc = prep["per_core"][c]
        in_maps.append({
            "featT": featT,
            "w_src": w_src, "w_dst": w_dst,
            "b_src_rep": b_src_rep, "b_dst_rep": b_dst_rep,
            "attn_rep": attn_rep, "iota_rep": iota_rep, "ident": ident,
            "w_fc": w_fc, "b_fc_rep": b_fc_rep,
            "sidx16w": pc["sidx16w"], "didx16w": pc["didx16w"],
            "dst_off": pc["dst_off"],
        })
    return in_maps


LAST_RESULTS = None


def kernel(**inputs) -> np.ndarray:
    global LAST_RESULTS
    cfg = GATCfg(
        use_act_lrelu=os.environ.get("GAT_ACT_LRELU", "0") == "1",
        use_f32r=os.environ.get("GAT_F32R", "0") == "1",
    )
    prep = preprocess(np.asarray(inputs["edge_index"]), cfg)
    nc = build(cfg, prep["K"], prep["NCHUNK"], prep["K_A"])
    in_maps = make_in_maps(inputs, cfg, prep)

    from concourse.bass_utils import run_bass_kernel_spmd
    res = run_bass_kernel_spmd(
        nc, in_maps, core_ids=list(range(cfg.NC)),
        trace=os.environ.get("GAT_TRACE", "0") == "1",
    )
    LAST_RESULTS = res

    out = np.empty((cfg.N, cfg.DOUT), np.float32)
    for c in range(cfg.NC):
        out[c * cfg.NPC:(c + 1) * cfg.NPC] = res.results[c]["z"][:cfg.NPC]
    return out
